# revision 1
# baseline (speedup 1.0000x reference)
"""DeltaNet-style block (nn_DeltaNet_31877247271438) on 8 trn2 NeuronCores.

Sharding: core c -> (batch b = c//2, pair-rank r = c%2).  Within a batch pair:
  - head-parallel: rank r owns heads {2r, 2r+1} (feature cols [512r, 512r+512))
  - cross-head mixes are K-split with pairwise collectives:
      * channel_mixer (folded with kernel_mix into one matrix Q): partial sums
        ReduceScatter'ed so each core receives its own heads' ms_out
      * fusion-MLP hidden is column-split; logits partials ReduceScatter'ed
      * bn features AllGather'ed (tiny)
  - the final Wo matmul partials are summed on the host.

Device layouts:
  hsT: (D, L) host-pretransposed.  q/k/v computed in (c x l) layout (free-dim
  shifts make the causal convs PE-friendly as diag-weight matmuls).  The delta
  rule runs with chunk=128 (mathematically equivalent to the reference's
  chunk=32) with (I - tril)^-1 factored as (I+B)(I+B^2) D via 32-block
  repeated squaring (strictly-lower triangular => nilpotent).
"""
import sys
sys.path.insert(0, '/opt/trn_rl_repo')

import numpy as np
import ml_dtypes

import concourse.bass as bass
import concourse.tile as tile
from concourse import bacc, mybir
from concourse.bass_utils import run_bass_kernel_spmd

F32 = mybir.dt.float32
F32R = mybir.dt.float32r
BF16 = mybir.dt.bfloat16
F16 = mybir.dt.float16
AF = mybir.ActivationFunctionType
ALU = mybir.AluOpType

B, L, D, H = 4, 2048, 1024, 4
d = 256          # per-head dim
C = 512          # channels owned per core (2 heads)
NLT = 16         # l-tiles of 128
NLW = 4          # l-windows of 512
NCH = 16         # delta chunks of 128
KQKV = 4         # qkv conv taps
MSK = (3, 15, 31)
NTAPS = sum(MSK)  # 49
PADV = 32
RG = [[0, 1], [2, 3], [4, 5], [6, 7]]


def r32(ap):
    return ap.bitcast(F32R)


def bc_mid(ap2, n):
    """[P, F] AP -> [P, n, F] with a 0-stride middle dim (free-dim bcast)."""
    assert len(ap2.ap) == 2
    return bass.AP(tensor=ap2.tensor, offset=ap2.offset,
                   ap=[ap2.ap[0], [0, n], ap2.ap[1]])


def build_program(debug=False):
    nc = bacc.Bacc("TRN2", target_bir_lowering=False, debug=False,
                   num_devices=8)

    io = {}
    io["hsT"] = nc.declare_dram_parameter("hsT", [D, L], F16, False)
    io["wq"] = nc.declare_dram_parameter("wq", [D, C], F16, False)
    io["wk"] = nc.declare_dram_parameter("wk", [D, C], F16, False)
    io["wv"] = nc.declare_dram_parameter("wv", [D, C], F16, False)
    io["wb"] = nc.declare_dram_parameter("wb", [D, 2], F16, False)
    io["cdiag"] = nc.declare_dram_parameter("cdiag", [3, 4, KQKV, 128, 128],
                                            F16, False)
    io["msdiag"] = nc.declare_dram_parameter("msdiag", [4, NTAPS, 128, 128],
                                             F16, False)
    io["qmix"] = nc.declare_dram_parameter("qmix", [12 * 128, D], F16, False)
    io["fw1h"] = nc.declare_dram_parameter("fw1h", [D, 1024], F16, False)
    io["fw1b"] = nc.declare_dram_parameter("fw1b", [16, 1024], F16, False)
    io["fb1"] = nc.declare_dram_parameter("fb1", [1024], F32, False)
    io["fw2"] = nc.declare_dram_parameter("fw2", [1024, 12], F16, False)
    io["b2o"] = nc.declare_dram_parameter("b2o", [128, 6], F32, False)
    io["wo"] = nc.declare_dram_parameter("wo", [C, D], F16, False)
    io["masks"] = nc.declare_dram_parameter("masks", [5, 128, 128], F32, False)
    io["onesrow"] = nc.declare_dram_parameter("onesrow", [1, 128], F32, False)
    io["onescol"] = nc.declare_dram_parameter("onescol", [128, 1], F32, False)
    io["ident16"] = nc.declare_dram_parameter("ident16", [128, 128], F16, False)
    io["out_part"] = nc.declare_dram_parameter("out_part", [L, D], F32, True)

    sc = {}
    sc["qkT_s"] = nc.dram_tensor("qkT_s", [3, 2, d, L], F16)
    sc["lc_s"] = nc.dram_tensor("lc_s", [2, 2, L, d], F16)
    sc["dout_s"] = nc.dram_tensor("dout_s", [L, C], F32)
    sc["cm_in"] = nc.dram_tensor("cm_in", [2, L, C], F32)
    sc["cm_out"] = nc.dram_tensor("cm_out", [L, C], F32)
    sc["bn_in"] = nc.dram_tensor("bn_in", [L, 8], F32)
    sc["bn_out"] = nc.dram_tensor("bn_out", [2, L, 8], F32)
    sc["lg_in"] = nc.dram_tensor("lg_in", [2, L, 6], F32)
    sc["lg_out"] = nc.dram_tensor("lg_out", [L, 6], F32)

    dbg = {}
    if debug:
        for nm, shp, dt_ in (("dbg_qT", [2, d, L], F16),
                             ("dbg_kbT", [2, d, L], F16),
                             ("dbg_dout", [L, C], F32),
                             ("dbg_cm", [L, C], F32),
                             ("dbg_vlc", [2, L, d], F16),
                             ("dbg_klc", [2, L, d], F16)):
            dbg[nm] = nc.declare_dram_parameter(nm, shp, dt_, True)

    with tile.TileContext(nc) as tc:
        _body(nc, tc, io, sc, dbg)
    nc.compile()
    return nc


def _body(nc, tc, io, sc, dbg):
    from contextlib import ExitStack
    ctx = ExitStack()
    with ctx:
        consts = ctx.enter_context(tc.tile_pool(name="consts", bufs=1))
        psum = ctx.enter_context(tc.tile_pool(name="psum", bufs=4,
                                              space="PSUM"))
        pab = ctx.enter_context(tc.tile_pool(name="pab", bufs=1))

        def ps(name="pst"):
            return psum.tile([128, 512], F32, tag="pst", name=name)

        def ps16(name="pst16"):
            return psum.tile([128, 512], F16, tag="pst", name=name)

        masks = consts.tile([128, 5, 128], F32)
        nc.sync.dma_start(out=masks,
                          in_=io["masks"][:].rearrange("m p f -> p m f"))
        ident = masks[:, 4, :]
        onesrow = consts.tile([1, 128], F32)
        nc.sync.dma_start(out=onesrow, in_=io["onesrow"][:])
        onescol = consts.tile([128, 1], F32)
        nc.sync.dma_start(out=onescol, in_=io["onescol"][:])
        ident16 = consts.tile([128, 128], F16)
        nc.sync.dma_start(out=ident16, in_=io["ident16"][:])
        onescol16 = consts.tile([128, 1], F16)
        nc.vector.memset(onescol16, 1.0)
        onesrow16 = consts.tile([1, 128], F16)
        nc.vector.memset(onesrow16, 1.0)

        beta_lp = consts.tile([128, NLT, 2], F32)
        S_sb = consts.tile([128, 2, 2, d], F32)   # (dk-part, h, dk-tile, dv)
        nc.vector.memset(S_sb, 0.0)
        S16 = consts.tile([128, 2, 2, d], F16)
        nc.vector.memset(S16, 0.0)
        eps6 = consts.tile([128, 1], F32)
        nc.vector.memset(eps6, 1e-6)
        eps5 = consts.tile([128, 1], F32)
        nc.vector.memset(eps5, 1e-5)

        vt_bf = pab.tile([128, 4, PADV + L], F16)    # conv-input v (fp16)
        nc.vector.memset(vt_bf[:, :, 0:PADV], 0.0)

        hsT_r = io["hsT"][:].rearrange("(kt p) l -> p kt l", p=128)

        # =================== PHASE A ======================================
        with tc.tile_pool(name="pa1", bufs=1) as pa1, \
             tc.tile_pool(name="pa2", bufs=2) as pa2, \
             tc.tile_pool(name="pas", bufs=3) as pas:
            hsT = pa1.tile([128, 8, L], F16)
            nc.sync.dma_start(out=hsT, in_=hsT_r)

            # ---- beta ----------------------------------------------------
            wb_sb = pa1.tile([128, 8, 2], F16)
            nc.sync.dma_start(
                out=wb_sb, in_=io["wb"][:].rearrange("(kt p) c -> p kt c",
                                                     p=128))
            # beta in l-partition form directly: psum[l, 2] per l-tile
            for lt in range(NLT):
                pb = ps("psb")
                pbv = pb[:, 0:2]
                for kt in range(8):
                    nc.tensor.matmul(pbv, hsT[:, kt, lt*128:(lt+1)*128],
                                     wb_sb[:, kt, :],
                                     start=(kt == 0), stop=(kt == 7))
                nc.scalar.activation(out=beta_lp[:, lt, :], in_=pbv,
                                     func=AF.Sigmoid)
            # row form per head (for the kb scale), via PE transposes
            brow = pa1.tile([1, 2, L], F32)
            for h in range(2):
                for lt in range(NLT):
                    pt = ps("psbt")
                    ptv = pt[0:1, 0:128]
                    nc.tensor.transpose(ptv, beta_lp[:, lt, h:h+1], ident)
                    nc.scalar.copy(out=brow[0:1, h, lt*128:(lt+1)*128],
                                   in_=ptv)

            # ---- q, k, v: proj -> conv -> silu (-> norm/stage) -----------
            for tnm, ti in (("q", 0), ("k", 1), ("v", 2)):
                PAD = 4
                w_sb = pa2.tile([128, 8, C], F16, tag="w_sb", bufs=1,
                                name=f"w_{tnm}")
                nc.sync.dma_start(
                    out=w_sb,
                    in_=io["w" + tnm][:].rearrange("(kt p) c -> p kt c",
                                                   p=128))
                cdg = pa2.tile([128, 4, KQKV, 128], F16, tag="cdg", bufs=1,
                               name=f"cdg_{tnm}")
                nc.sync.dma_start(
                    out=cdg,
                    in_=io["cdiag"][ti].rearrange("ct tap p f -> p ct tap f"))
                if tnm != "v":
                    xc = pa1.tile([128, 4, 4 + L], F16, tag="xc",
                                  name=f"xc_{tnm}")
                    nc.vector.memset(xc[:, :, 0:4], 0.0)

                for ct in range(4):
                    xp = pa2.tile([128, 4 + L], F16, tag="xp",
                                  name=f"xp_{tnm}{ct}")
                    nc.vector.memset(xp[:, 0:4], 0.0)
                    for lw in range(NLW):
                        pp = ps("psp")
                        for kt in range(8):
                            nc.tensor.matmul(
                                pp, w_sb[:, kt, ct*128:(ct+1)*128],
                                hsT[:, kt, lw*512:(lw+1)*512],
                                start=(kt == 0), stop=(kt == 7))
                        nc.scalar.copy(out=xp[:, 4+lw*512:4+(lw+1)*512],
                                       in_=pp)
                    if tnm == "v":
                        vt_ct = pa2.tile([128, L], F16, tag="vt_ct",
                                         name=f"vt{ct}")
                        h, dt = ct // 2, ct % 2
                        for lw in range(NLW):
                            pc = ps("psc")
                            for dd in range(KQKV):
                                off = 4 + lw*512 - dd
                                nc.tensor.matmul(
                                    pc, cdg[:, ct, dd, :],
                                    xp[:, off:off+512],
                                    start=(dd == 0), stop=(dd == KQKV-1))
                            nc.scalar.activation(
                                out=vt_ct[:, lw*512:(lw+1)*512],
                                in_=pc, func=AF.Silu)
                        nc.scalar.copy(out=vt_bf[:, ct, PADV:], in_=vt_ct)
                        for lt in range(NLT):
                            ptr = ps16("psvt")
                            ptv = ptr[:, 0:128]
                            nc.tensor.transpose(
                                ptv, vt_ct[:, lt*128:(lt+1)*128], ident16)
                            st = pas.tile([128, 128], F16, tag="st_t", bufs=1,
                                          name="st_v")
                            nc.scalar.copy(out=st, in_=ptv)
                            nc.sync.dma_start(
                                out=sc["lc_s"][1, h, lt*128:(lt+1)*128,
                                               dt*128:(dt+1)*128],
                                in_=st)
                    else:
                        for lw in range(NLW):
                            pc = ps("psc")
                            for dd in range(KQKV):
                                off = 4 + lw*512 - dd
                                nc.tensor.matmul(
                                    pc, cdg[:, ct, dd, :],
                                    xp[:, off:off+512],
                                    start=(dd == 0), stop=(dd == KQKV-1))
                            nc.scalar.activation(
                                out=xc[:, ct, PAD+lw*512:PAD+(lw+1)*512],
                                in_=pc, func=AF.Silu)
                if tnm == "v":
                    continue

                # ---- l2norm (q, k), stage qT/kT (+ kbT, klc) -------------
                dst = sc["qkT_s"][0] if tnm == "q" else sc["qkT_s"][1]
                for h in range(2):
                    for lw in range(NLW):
                        lsl = slice(PAD+lw*512, PAD+(lw+1)*512)
                        pss = ps("psss")
                        pssv = pss[0:1, :]
                        for i, ct in enumerate((2*h, 2*h+1)):
                            sq = pas.tile([128, 512], F16, tag="sq", bufs=2,
                                          name="sq")
                            nc.vector.tensor_tensor(
                                out=sq, in0=xc[:, ct, lsl],
                                in1=xc[:, ct, lsl], op=ALU.mult)
                            nc.tensor.matmul(pssv, onescol16, sq,
                                             start=(i == 0), stop=(i == 1))
                        sr = pas.tile([1, 512], F32, tag="sr", bufs=1,
                                      name="sr")
                        nc.scalar.activation(out=sr, in_=pssv, func=AF.Sqrt,
                                             bias=eps6[0:1, :])
                        srt = pas.tile([1, 512], F16, tag="srt", bufs=1,
                                       name="srt")
                        with nc.allow_low_precision("l2norm scale fp16"):
                            nc.vector.reciprocal(out=srt, in_=sr)
                        pbc = ps("psbc")
                        nc.tensor.matmul(pbc, onesrow16, srt,
                                         start=True, stop=True)
                        for dt in range(2):
                            ct = 2*h + dt
                            xn = pas.tile([128, 512], F16, tag="xn", bufs=1,
                                          name="xn")
                            nc.vector.tensor_tensor(
                                out=xn, in0=xc[:, ct, lsl], in1=pbc,
                                op=ALU.mult)
                            nc.sync.dma_start(
                                out=dst[h, dt*128:(dt+1)*128,
                                        lw*512:(lw+1)*512],
                                in_=xn)
                            if tnm == "k":
                                nc.vector.tensor_copy(
                                    out=xc[:, ct, lsl], in_=xn)
                        if tnm == "k":
                            # xc now holds normalized k, so scale by beta only
                            pbb = ps("psbb")
                            nc.tensor.matmul(
                                pbb, onesrow,
                                brow[0:1, h, lw*512:(lw+1)*512],
                                start=True, stop=True)
                            for dt in range(2):
                                ct = 2*h + dt
                                xb = pas.tile([128, 512], F16, tag="xb", bufs=2,
                                              name="xb")
                                nc.vector.tensor_tensor(
                                    out=xb, in0=xc[:, ct, lsl], in1=pbb,
                                    op=ALU.mult)
                                nc.sync.dma_start(
                                    out=sc["qkT_s"][2, h,
                                                    dt*128:(dt+1)*128,
                                                    lw*512:(lw+1)*512],
                                    in_=xb)
                if tnm == "k":
                    for ct in range(4):
                        h, dt = ct // 2, ct % 2
                        for lt in range(NLT):
                            ptr = ps16("pskt")
                            ptv = ptr[:, 0:128]
                            nc.tensor.transpose(
                                ptv, xc[:, ct, PAD+lt*128:PAD+(lt+1)*128],
                                ident16)
                            st = pas.tile([128, 128], F16, tag="st_t", bufs=1,
                                          name="st_k")
                            nc.scalar.copy(out=st, in_=ptv)
                            nc.sync.dma_start(
                                out=sc["lc_s"][0, h, lt*128:(lt+1)*128,
                                               dt*128:(dt+1)*128],
                                in_=st)

        # =================== PHASE B ======================================
        with tc.tile_pool(name="pb1", bufs=1) as pb1, \
             tc.tile_pool(name="pbm", bufs=2) as pbm, \
             tc.tile_pool(name="pbs", bufs=3) as pbs, \
             tc.tile_pool(name="pdi", bufs=2) as pdi, \
             tc.tile_pool(name="pdc", bufs=1) as pdc, \
             tc.tile_pool(name="pdw", bufs=1) as pdw:
            # ---- B1: multiscale convs ------------------------------------
            y_bf = pb1.tile([128, 12, L], F16)
            for ct in range(4):
                msd = pbm.tile([128, NTAPS, 128], F16, tag="msd",
                               name=f"msd{ct}")
                nc.sync.dma_start(
                    out=msd,
                    in_=io["msdiag"][ct].rearrange("j p f -> p j f"))
                for lw in range(NLW):
                    base = 0
                    for si, ks in enumerate(MSK):
                        py = ps("psy")
                        for dd in range(ks):
                            off = PADV + lw*512 - dd
                            nc.tensor.matmul(py, msd[:, base+dd, :],
                                             vt_bf[:, ct, off:off+512],
                                             start=(dd == 0),
                                             stop=(dd == ks-1))
                        nc.scalar.copy(
                            out=y_bf[:, si*4+ct, lw*512:(lw+1)*512], in_=py)
                        base += ks

            # ---- B2: folded kernel_mix @ channel_mixer partial -----------
            qmix_sb = pb1.tile([128, 12, D], F16)
            nc.sync.dma_start(
                out=qmix_sb,
                in_=io["qmix"][:].rearrange("(kt p) o -> p kt o", p=128))
            for lt in range(NLT):
                for oh in range(2):
                    pq = ps("psq")
                    for kt in range(12):
                        nc.tensor.matmul(
                            pq, y_bf[:, kt, lt*128:(lt+1)*128],
                            qmix_sb[:, kt, oh*512:(oh+1)*512],
                            start=(kt == 0), stop=(kt == 11))
                    cmt = pbs.tile([128, 512], F32, tag="cmt", name="cmt")
                    nc.scalar.copy(out=cmt, in_=pq)
                    nc.sync.dma_start(
                        out=sc["cm_in"][oh, lt*128:(lt+1)*128, :], in_=cmt)
            nc.gpsimd.collective_compute(
                "ReduceScatter", ALU.add, replica_groups=RG,
                ins=[sc["cm_in"][:]], outs=[sc["cm_out"][:]])

            # ---- B3: delta rule ------------------------------------------
            qkT_r = sc["qkT_s"][:].rearrange(
                "t h (kt p) l -> p t kt h l", p=128)

            for ci in range(NCH):
                lsl = slice(ci*128, (ci+1)*128)
                qkTt = pdi.tile([128, 3, 2, 2, 128], F16, tag="qkTt",
                                name="qkTt")
                for ti in range(3):
                    for h in range(2):
                        nc.sync.dma_start(out=qkTt[:, ti, :, h, :],
                                          in_=qkT_r[:, ti, :, h, lsl])
                qTt = qkTt[:, 0]
                kTt = qkTt[:, 1]
                kbTt = qkTt[:, 2]
                lct = pdi.tile([128, 2, 2, d], F16, tag="lct", name="lct")
                for ti in range(2):
                    nc.sync.dma_start(
                        out=lct[:, ti],
                        in_=sc["lc_s"][ti, :, lsl, :].rearrange(
                            "h p e -> p h e"))
                klct = lct[:, 0]
                vlct = lct[:, 1]

                def blk(name, bufs=1):
                    return pdc.tile([128, 2, 128], F16, tag=name, name=name,
                                    bufs=bufs)

                def mm2(pt, lhs_fn, rhs_fn, n_k=1, rr=False):
                    """per-head matmuls into pt[:, h, :]."""
                    for h in range(2):
                        for kt in range(n_k):
                            a, b_ = lhs_fn(h, kt), rhs_fn(h, kt)
                            if rr:
                                a, b_ = a, b_
                            nc.tensor.matmul(pt[:, h, :], a, b_,
                                             start=(kt == 0),
                                             stop=(kt == n_k-1))

                pG = psum.tile([128, 2, 128], F32, tag="pd", bufs=3, name="pG")
                mm2(pG, lambda h, kt: kbTt[:, kt, h, :],
                    lambda h, kt: kTt[:, kt, h, :], n_k=2)
                pGT = psum.tile([128, 2, 128], F32, tag="pd", bufs=3, name="pGT")
                mm2(pGT, lambda h, kt: kTt[:, kt, h, :],
                    lambda h, kt: kbTt[:, kt, h, :], n_k=2)

                Td, To, TdT = blk("Td"), blk("To"), blk("TdT")
                nc.vector.tensor_tensor(out=Td, in0=pG,
                                        in1=bc_mid(masks[:, 0, :], 2),
                                        op=ALU.mult)
                nc.vector.tensor_tensor(out=To, in0=pG,
                                        in1=bc_mid(masks[:, 1, :], 2),
                                        op=ALU.mult)
                nc.vector.tensor_tensor(out=TdT, in0=pGT,
                                        in1=bc_mid(masks[:, 2, :], 2),
                                        op=ALU.mult)

                # squaring chain (32-block diag): T2,T2T,T4,T4T,T8,T8T,T16
                chain = {}
                specs = [("T2", "TdT", "Td"), ("T2T", "Td", "TdT"),
                         ("T4", "T2T", "T2"), ("T4T", "T2", "T2T"),
                         ("T8", "T4T", "T4"), ("T8T", "T4", "T4T"),
                         ("T16", "T8T", "T8")]
                base = {"Td": Td, "TdT": TdT}
                for nm, ln, rn in specs:
                    src = {**base, **chain}
                    pq2 = psum.tile([128, 2, 128], F32, tag="pd", bufs=3, name="pq2")
                    mm2(pq2, lambda h, kt, a=src[ln]: a[:, h, :],
                        lambda h, kt, b_=src[rn]: b_[:, h, :])
                    chain[nm] = blk(nm)
                    nc.scalar.copy(out=chain[nm], in_=pq2)

                # product chain for D^T
                MT = blk("MT", bufs=2)
                nc.vector.tensor_tensor(out=MT, in0=TdT,
                                        in1=bc_mid(ident, 2), op=ALU.add)
                for nm in ("T2", "T4", "T8", "T16"):
                    pm = psum.tile([128, 2, 128], F32, tag="pd", bufs=3, name="pm")
                    mm2(pm, lambda h, kt, a=chain[nm]: a[:, h, :],
                        lambda h, kt, b_=MT: b_[:, h, :])
                    MTn = blk("MT", bufs=2)
                    nc.vector.scalar_tensor_tensor(
                        out=MTn, in0=pm, scalar=1.0, in1=MT,
                        op0=ALU.mult, op1=ALU.add)
                    MT = MTn
                DT = MT

                pB = psum.tile([128, 2, 128], F32, tag="pd", bufs=3, name="pB")
                mm2(pB, lambda h, kt: DT[:, h, :], lambda h, kt: To[:, h, :])
                Bm = blk("Bm")
                nc.scalar.copy(out=Bm, in_=pB)
                pBT = psum.tile([128, 2, 128], F32, tag="pd", bufs=3, name="pBT")
                mm2(pBT, lambda h, kt: To[:, h, :], lambda h, kt: DT[:, h, :])
                BT = blk("BT")
                nc.scalar.copy(out=BT, in_=pBT)
                pB2 = psum.tile([128, 2, 128], F32, tag="pd", bufs=3, name="pB2")
                mm2(pB2, lambda h, kt: Bm[:, h, :], lambda h, kt: BT[:, h, :])
                B2T = blk("B2T")
                nc.scalar.copy(out=B2T, in_=pB2)

                pA = psum.tile([128, 2, 128], F32, tag="pd", bufs=3, name="pA")
                mm2(pA, lambda h, kt: kTt[:, kt, h, :],
                    lambda h, kt: qTt[:, kt, h, :], n_k=2)
                aT = blk("aT")
                nc.vector.tensor_tensor(out=aT, in0=pA,
                                        in1=bc_mid(masks[:, 3, :], 2),
                                        op=ALU.mult)

                vb = pdw.tile([128, 2, d], F16, tag="vb", name="vb")
                kbl = pdw.tile([128, 2, d], F16, tag="kbl", name="kbl")
                for h in range(2):
                    nc.vector.tensor_scalar_mul(vb[:, h, :], vlct[:, h, :],
                                                beta_lp[:, ci, h:h+1])
                    nc.vector.tensor_scalar_mul(kbl[:, h, :], klct[:, h, :],
                                                beta_lp[:, ci, h:h+1])

                uw = {}
                for xnm, xt in (("u", vb), ("w", kbl)):
                    px1 = psum.tile([128, 2, d], F32, tag="pd", bufs=3, name="px1")
                    mm2(px1, lambda h, kt: DT[:, h, :],
                        lambda h, kt: xt[:, h, :], rr=True)
                    x1 = pdw.tile([128, 2, d], F16, tag="x1", name="x1")
                    nc.scalar.copy(out=x1, in_=px1)
                    py1 = psum.tile([128, 2, d], F32, tag="pd", bufs=3, name="py1")
                    mm2(py1, lambda h, kt: B2T[:, h, :],
                        lambda h, kt: x1[:, h, :], rr=True)
                    y1 = pdw.tile([128, 2, d], F16, tag="y1", name="y1")
                    nc.vector.scalar_tensor_tensor(
                        out=y1, in0=py1, scalar=1.0, in1=x1,
                        op0=ALU.mult, op1=ALU.add)
                    pu = psum.tile([128, 2, d], F32, tag="pd", bufs=3, name="pu")
                    mm2(pu, lambda h, kt: BT[:, h, :],
                        lambda h, kt: y1[:, h, :], rr=True)
                    ut = pdw.tile([128, 2, d], F16, tag=f"uw_{xnm}",
                                  name=f"uw_{xnm}")
                    nc.vector.scalar_tensor_tensor(
                        out=ut, in0=pu, scalar=1.0, in1=y1,
                        op0=ALU.mult, op1=ALU.add)
                    uw[xnm] = ut

                # wT via PE transpose
                wTt = pdw.tile([128, 2, 2, 128], F16, tag="wTt", name="wTt")
                for kt in range(2):
                    ptw = psum.tile([128, 2, 128], F16, tag="pd", bufs=3, name="ptw")
                    for h in range(2):
                        nc.tensor.transpose(
                            ptw[:, h, :],
                            uw["w"][:, h, kt*128:(kt+1)*128], ident16)
                    nc.scalar.copy(out=wTt[:, kt, :, :], in_=ptw)

                pup = psum.tile([128, 2, d], F32, tag="pd", bufs=3, name="pup")
                for h in range(2):
                    for kt in range(2):
                        nc.tensor.matmul(pup[:, h, :],
                                         wTt[:, kt, h, :],
                                         S16[:, h, kt, :],
                                         start=(kt == 0), stop=(kt == 1))
                upr = pdw.tile([128, 2, d], F16, tag="upr", name="upr")
                nc.vector.scalar_tensor_tensor(
                    out=upr, in0=pup, scalar=-1.0, in1=uw["u"],
                    op0=ALU.mult, op1=ALU.add)

                po = psum.tile([128, 2, d], F32, tag="pd", bufs=3, name="po")
                for h in range(2):
                    for kt in range(2):
                        nc.tensor.matmul(po[:, h, :], qTt[:, kt, h, :],
                                         S16[:, h, kt, :],
                                         start=(kt == 0), stop=False)
                    nc.tensor.matmul(po[:, h, :], aT[:, h, :],
                                     upr[:, h, :],
                                     start=False, stop=True)
                ot = pdw.tile([128, 2, d], F32, tag="ot", name="ot")
                nc.scalar.copy(out=ot, in_=po)
                nc.sync.dma_start(out=sc["dout_s"][lsl, :],
                                  in_=ot.rearrange("p h e -> p (h e)"))

                for h in range(2):
                    pdS = psum.tile([128, 2, d], F32, tag="pd", bufs=3,
                                    name=f"pdS{h}")
                    for mt in range(2):
                        nc.tensor.matmul(pdS[:, mt, :],
                                         klct[:, h, mt*128:(mt+1)*128],
                                         upr[:, h, :],
                                         start=True, stop=True)
                    nc.vector.tensor_tensor(out=S_sb[:, h], in0=S_sb[:, h],
                                            in1=pdS, op=ALU.add)
                    nc.scalar.copy(out=S16[:, h], in_=S_sb[:, h])

        # =================== PHASE C ======================================
        with tc.tile_pool(name="pc1", bufs=1) as pc1, \
             tc.tile_pool(name="pc2", bufs=2) as pc2, \
             tc.tile_pool(name="pcs", bufs=3) as pcs:
            # ---- bn features ---------------------------------------------
            bn_sb = pc1.tile([128, NLT, 8], F32)
            nc.vector.memset(bn_sb, 0.0)
            for lt in range(NLT):
                vl = pcs.tile([128, 2, d], F16, tag="vl", name="vl")
                nc.sync.dma_start(
                    out=vl,
                    in_=sc["lc_s"][1, :, lt*128:(lt+1)*128, :].rearrange(
                        "h p e -> p h e"))
                ms_t = pcs.tile([128, 2, d], F32, tag="ms_t", name="ms_t")
                nc.sync.dma_start(
                    out=ms_t,
                    in_=sc["cm_out"][lt*128:(lt+1)*128, :].rearrange(
                        "p (h e) -> p h e", e=d))
                do_t = pcs.tile([128, 2, d], F32, tag="do_t", name="do_t")
                nc.sync.dma_start(
                    out=do_t,
                    in_=sc["dout_s"][lt*128:(lt+1)*128, :].rearrange(
                        "p (h e) -> p h e", e=d))
                for si, src in enumerate((ms_t, do_t, vl)):
                    nc.vector.tensor_reduce(
                        out=bn_sb[:, lt, 2*si:2*si+2], in_=src,
                        axis=mybir.AxisListType.X, op=ALU.add,
                        apply_absolute_value=True)
            nc.sync.dma_start(
                out=sc["bn_in"][:].rearrange("(lt p) c -> p lt c", p=128),
                in_=bn_sb)
            nc.gpsimd.collective_compute(
                "AllGather", ALU.bypass, replica_groups=RG,
                ins=[sc["bn_in"][:]], outs=[sc["bn_out"][:]])

            bnT = [pc1.tile([8, L], F16, name=f"bnT{m}") for m in range(2)]
            for m in range(2):
                bng = pc2.tile([128, NLT, 8], F32, tag="bng", name=f"bng{m}")
                nc.sync.dma_start(
                    out=bng,
                    in_=sc["bn_out"][m].rearrange("(lt p) c -> p lt c",
                                                  p=128))
                for lt in range(NLT):
                    ptb = ps("ptb")
                    ptbv = ptb[0:8, 0:128]
                    nc.tensor.transpose(ptbv, bng[:, lt, :], ident)
                    nc.scalar.mul(out=bnT[m][:, lt*128:(lt+1)*128],
                                  in_=ptbv, mul=1.0/d)

            # ---- fusion MLP ----------------------------------------------
            fb1_sb = pc1.tile([128, 8], F32)
            nc.sync.dma_start(out=fb1_sb,
                              in_=io["fb1"][:].rearrange("(m p) -> p m",
                                                         p=128))
            fw1b_sb = pc1.tile([8, 2, 1024], F16)
            nc.sync.dma_start(
                out=fw1b_sb,
                in_=io["fw1b"][:].rearrange("(m p) c -> p m c", p=8))
            hdnT = pc1.tile([128, 8, L], F16)
            fw1h_r = io["fw1h"][:].rearrange("(kt p) m -> p kt m", p=128)
            for lw in range(NLW):
                hst = pc1.tile([128, 8, 512], F16, tag="hst", name="hst")
                nc.sync.dma_start(out=hst, in_=hsT_r[:, :, lw*512:(lw+1)*512])
                for mt in range(8):
                    fwt = pc2.tile([128, 8, 128], F16, tag="fwt",
                                   name=f"fwt{mt}")
                    nc.sync.dma_start(out=fwt,
                                      in_=fw1h_r[:, :, mt*128:(mt+1)*128])
                    ph = ps("psh")
                    for kt in range(8):
                        nc.tensor.matmul(ph, fwt[:, kt, :],
                                         hst[:, kt, :],
                                         start=(kt == 0), stop=False)
                    for m in range(2):
                        nc.tensor.matmul(ph,
                                         fw1b_sb[:, m, mt*128:(mt+1)*128],
                                         bnT[m][:, lw*512:(lw+1)*512],
                                         start=False, stop=(m == 1))
                    nc.scalar.activation(out=hdnT[:, mt, lw*512:(lw+1)*512],
                                         in_=ph, func=AF.Gelu,
                                         bias=fb1_sb[:, mt:mt+1])

            fw2_sb = pc1.tile([128, 8, 12], F16)
            nc.sync.dma_start(
                out=fw2_sb,
                in_=io["fw2"][:].rearrange("(kt p) c -> p kt c", p=128))
            lg_sb = pc1.tile([128, NLT, 12], F32)
            for lt in range(NLT):
                pl = ps("psl")
                plv = pl[:, 0:12]
                for kt in range(8):
                    nc.tensor.matmul(plv, hdnT[:, kt, lt*128:(lt+1)*128],
                                     fw2_sb[:, kt, :],
                                     start=(kt == 0), stop=(kt == 7))
                nc.scalar.copy(out=lg_sb[:, lt, :], in_=plv)
            for m in range(2):
                nc.sync.dma_start(
                    out=sc["lg_in"][m].rearrange("(lt p) c -> p lt c", p=128),
                    in_=lg_sb[:, :, m*6:(m+1)*6])
            nc.gpsimd.collective_compute(
                "ReduceScatter", ALU.add, replica_groups=RG,
                ins=[sc["lg_in"][:]], outs=[sc["lg_out"][:]])

            # ---- softmax gates -------------------------------------------
            b2_sb = pc1.tile([128, 6], F32)
            nc.sync.dma_start(out=b2_sb, in_=io["b2o"][:])
            lgo = pc1.tile([128, NLT, 2, 3], F32)
            nc.sync.dma_start(
                out=lgo,
                in_=sc["lg_out"][:].rearrange("(lt p) (h e) -> p lt h e",
                                              p=128, e=3))
            nc.vector.tensor_tensor(
                out=lgo, in0=lgo,
                in1=bass.AP(tensor=b2_sb.tensor, offset=b2_sb.offset,
                            ap=[b2_sb.ap[0], [0, NLT], [3, 2], [1, 3]]),
                op=ALU.add)
            rmax = pc1.tile([128, NLT, 2], F32)
            nc.vector.tensor_reduce(out=rmax, in_=lgo,
                                    axis=mybir.AxisListType.X, op=ALU.max)
            nc.vector.tensor_tensor(
                out=lgo, in0=lgo,
                in1=rmax[:, :, :, None].to_broadcast([128, NLT, 2, 3]),
                op=ALU.subtract)
            nc.scalar.activation(out=lgo, in_=lgo, func=AF.Exp)
            rsum = pc1.tile([128, NLT, 2], F32)
            nc.vector.tensor_reduce(out=rsum, in_=lgo,
                                    axis=mybir.AxisListType.X, op=ALU.add)
            nc.vector.reciprocal(out=rsum, in_=rsum)
            nc.vector.tensor_tensor(
                out=lgo, in0=lgo,
                in1=rsum[:, :, :, None].to_broadcast([128, NLT, 2, 3]),
                op=ALU.mult)

            # ---- gate mix + RMSNorm + Wo ---------------------------------
            wo_sb = pc1.tile([128, 4, D], F16)
            nc.sync.dma_start(
                out=wo_sb,
                in_=io["wo"][:].rearrange("(kt p) n -> p kt n", p=128))
            for lt in range(NLT):
                vl = pcs.tile([128, 2, d], F16, tag="vl", name="vl2")
                nc.sync.dma_start(
                    out=vl,
                    in_=sc["lc_s"][1, :, lt*128:(lt+1)*128, :].rearrange(
                        "h p e -> p h e"))
                msv = pcs.tile([128, 2, d], F32, tag="ms_t", name="ms_t2")
                nc.sync.dma_start(
                    out=msv,
                    in_=sc["cm_out"][lt*128:(lt+1)*128, :].rearrange(
                        "p (h e) -> p h e", e=d))
                dov = pcs.tile([128, 2, d], F32, tag="do_t", name="do_t2")
                nc.sync.dma_start(
                    out=dov,
                    in_=sc["dout_s"][lt*128:(lt+1)*128, :].rearrange(
                        "p (h e) -> p h e", e=d))
                o_t = pcs.tile([128, 2, d], F16, tag="o_t", name="o_t")
                ssq = pcs.tile([128, 2], F32, tag="ssq", name="ssq")
                scr = pcs.tile([128, d], F32, tag="scr", name="scr")
                for h in range(2):
                    nc.vector.tensor_scalar_mul(o_t[:, h, :], msv[:, h, :],
                                                lgo[:, lt, h, 0:1])
                    nc.vector.scalar_tensor_tensor(
                        out=o_t[:, h, :], in0=dov[:, h, :],
                        scalar=lgo[:, lt, h, 1:2], in1=o_t[:, h, :],
                        op0=ALU.mult, op1=ALU.add)
                    nc.vector.scalar_tensor_tensor(
                        out=o_t[:, h, :], in0=vl[:, h, :],
                        scalar=lgo[:, lt, h, 2:3], in1=o_t[:, h, :],
                        op0=ALU.mult, op1=ALU.add)
                    nc.scalar.activation(out=scr, in_=o_t[:, h, :],
                                         func=AF.Square,
                                         accum_out=ssq[:, h:h+1])
                nc.scalar.activation(out=ssq, in_=ssq, func=AF.Sqrt,
                                     scale=1.0/d, bias=eps5)
                nc.vector.reciprocal(out=ssq, in_=ssq)
                for h in range(2):
                    nc.vector.tensor_scalar_mul(o_t[:, h, :], o_t[:, h, :],
                                                ssq[:, h:h+1])
                oT = pcs.tile([128, 4, 128], F16, tag="oT", name="oT")
                for ct in range(4):
                    h, dt = ct // 2, ct % 2
                    pto = ps16("psto")
                    ptov = pto[:, 0:128]
                    nc.tensor.transpose(ptov,
                                        o_t[:, h, dt*128:(dt+1)*128], ident16)
                    nc.scalar.copy(out=oT[:, ct, :], in_=ptov)
                orow = pcs.tile([128, D], F32, tag="orow", name="orow")
                for nh in range(2):
                    pw = ps("psw")
                    for ct in range(4):
                        nc.tensor.matmul(pw, oT[:, ct, :],
                                         wo_sb[:, ct,
                                                   nh*512:(nh+1)*512],
                                         start=(ct == 0), stop=(ct == 3))
                    nc.scalar.copy(out=orow[:, nh*512:(nh+1)*512], in_=pw)
                nc.sync.dma_start(out=io["out_part"][lt*128:(lt+1)*128, :],
                                  in_=orow)

        if dbg:
            nc.sync.dma_start(out=dbg["dbg_qT"][:], in_=sc["qkT_s"][0])
            nc.sync.dma_start(out=dbg["dbg_kbT"][:], in_=sc["qkT_s"][2])
            nc.sync.dma_start(out=dbg["dbg_dout"][:], in_=sc["dout_s"][:])
            nc.sync.dma_start(out=dbg["dbg_cm"][:], in_=sc["cm_out"][:])
            nc.sync.dma_start(out=dbg["dbg_vlc"][:], in_=sc["lc_s"][1])
            nc.sync.dma_start(out=dbg["dbg_klc"][:], in_=sc["lc_s"][0])


# ======================= host side =======================================

def _diag_tiles(w_own, taps, out_dtype):
    """w_own: (C, k) conv weights for this core's channels.
    Returns (4, k, 128, 128) diag tiles; tap dd uses column k-1-dd."""
    k = w_own.shape[1]
    out = np.zeros((4, k, 128, 128), dtype=out_dtype)
    for ct in range(4):
        for dd in range(k):
            np.fill_diagonal(out[ct, dd], w_own[ct*128:(ct+1)*128, k-1-dd])
    return out


def _host_inputs(inputs):
    hs = np.asarray(inputs["hidden_states"], np.float32)
    Wq = np.asarray(inputs["Wq"], np.float32)
    Wk = np.asarray(inputs["Wk"], np.float32)
    Wv = np.asarray(inputs["Wv"], np.float32)
    Wb = np.asarray(inputs["Wb"], np.float32)
    cq = np.asarray(inputs["conv_q_w"], np.float32)
    ck = np.asarray(inputs["conv_k_w"], np.float32)
    cv = np.asarray(inputs["conv_v_w"], np.float32)
    w3 = np.asarray(inputs["ms_w3"], np.float32)
    w15 = np.asarray(inputs["ms_w15"], np.float32)
    w31 = np.asarray(inputs["ms_w31"], np.float32)
    kmix = np.asarray(inputs["kernel_mix_w"], np.float32)
    cmix = np.asarray(inputs["channel_mixer_w"], np.float32)
    fw1 = np.asarray(inputs["fusion_w1"], np.float32)
    fb1 = np.asarray(inputs["fusion_b1"], np.float32)
    fw2 = np.asarray(inputs["fusion_w2"], np.float32)
    fb2 = np.asarray(inputs["fusion_b2"], np.float32)
    onw = np.asarray(inputs["o_norm_w"], np.float32)
    Wo = np.asarray(inputs["Wo"], np.float32)

    # combined kernel_mix -> channel_mixer matrix Q: (3D, D)
    Q = np.zeros((3 * D, D), np.float32)
    for h in range(H):
        Q[h*3*d:(h+1)*3*d] = kmix @ cmix[h*d:(h+1)*d]

    masks = np.zeros((5, 128, 128), np.float32)
    i_, j_ = np.mgrid[0:128, 0:128]
    blk = (i_ // 32) == (j_ // 32)
    masks[0] = -((i_ > j_) & blk).astype(np.float32)
    masks[1] = -((i_ > j_) & ~blk).astype(np.float32)
    masks[2] = -((j_ > i_) & blk).astype(np.float32)
    masks[3] = (j_ >= i_).astype(np.float32)
    masks[4] = np.eye(128, dtype=np.float32)

    Wo_s = Wo * np.tile(onw, H)[:, None]

    in_maps = []
    for c in range(8):
        b, r = divmod(c, 2)
        cs = slice(C*r, C*(r+1))
        qmix = np.concatenate(
            [Q[1024*s + C*r: 1024*s + C*r + C] for s in range(3)], 0)
        msdiag = np.concatenate(
            [_diag_tiles(w[cs], w.shape[1], np.float16)[:, None]
             .reshape(4, -1, 128, 128)
             for w in (w3, w15, w31)], axis=1)
        cdiag = np.stack([_diag_tiles(w[cs], KQKV, np.float16)
                          for w in (cq, ck, cv)], 0)
        fw1b = np.zeros((16, 1024), np.float32)
        for m in range(2):
            for src in range(3):
                for h_ in range(2):
                    fw1b[m*8 + src*2 + h_] = \
                        fw1[D + src*4 + 2*m + h_, 1024*r:1024*(r+1)]
        fw2p = np.zeros((1024, 12), np.float32)
        b2o = np.zeros((6,), np.float32)
        for jm in range(2):
            for h_ in range(2):
                for br in range(3):
                    gcol = (2*jm + h_)*3 + br
                    fw2p[:, jm*6 + h_*3 + br] = fw2[1024*r:1024*(r+1), gcol]
        for h_ in range(2):
            for br in range(3):
                b2o[h_*3 + br] = fb2[(2*r + h_)*3 + br]
        m = {
            "hsT": np.ascontiguousarray(hs[b].T).astype(np.float16),
            "wq": np.ascontiguousarray(Wq[:, cs]).astype(np.float16),
            "wk": np.ascontiguousarray(Wk[:, cs]).astype(np.float16),
            "wv": np.ascontiguousarray(Wv[:, cs]).astype(np.float16),
            "wb": np.ascontiguousarray(Wb[:, 2*r:2*r+2]).astype(np.float16),
            "cdiag": cdiag,
            "msdiag": np.ascontiguousarray(msdiag),
            "qmix": qmix.astype(np.float16),
            "fw1h": np.ascontiguousarray(
                fw1[:D, 1024*r:1024*(r+1)]).astype(np.float16),
            "fw1b": fw1b.astype(np.float16),
            "fb1": np.ascontiguousarray(fb1[1024*r:1024*(r+1)]),
            "fw2": fw2p.astype(np.float16),
            "b2o": np.tile(b2o, (128, 1)),
            "wo": np.ascontiguousarray(Wo_s[cs, :]).astype(np.float16),
            "masks": masks,
            "onesrow": np.ones((1, 128), np.float32),
            "onescol": np.ones((128, 1), np.float32),
            "ident16": np.eye(128, dtype=np.float16),
        }
        in_maps.append(m)
    return in_maps


_PROG = {}


def _get_program(debug=False):
    key = bool(debug)
    if key not in _PROG:
        _PROG[key] = build_program(debug=debug)
    return _PROG[key]


def run(inputs, debug=False, **kw):
    nc = _get_program(debug=debug)
    in_maps = _host_inputs(inputs)
    res = run_bass_kernel_spmd(nc, in_maps, list(range(8)), **kw)
    return res


def kernel(**inputs):
    res = run(inputs)
    out = np.zeros((B, L, D), np.float32)
    for b in range(B):
        out[b] = res.results[2*b]["out_part"] + res.results[2*b+1]["out_part"]
    return out


if __name__ == "__main__":
    nc = build_program()
    print("program built ok")

